# revision 1
# baseline (speedup 1.0000x reference)
"""Grouped-query attention kernel for 8 Trainium2 NeuronCores.

Problem (hardcoded): x [2, 512, 16, 16, 16] f32, Wq/Wk/Wv/Wo [512, 512],
biases [512]. G=4 heads of dim 128, N=4096 tokens. out = x + Wo@attn.

Sharding: one (batch, group) pair per core -> 8 cores, no cross-core
communication. Each core computes its group's Q/K/V projections, the
full 4096x4096 attention for its (b, g), and a partial output
projection Wo[:, g_cols] @ O_g -> [512, 4096]. Host sums the 4 partials
per batch and adds the residual + bo.

Device-side layout (per core):
  - xf (x[b] as [512, 4096]) bf16, 4 chunks of [128, 4096] in SBUF
  - Q, K: [128(gs), 4096] bf16; V^T: [128(keys-chunk), 32*128] bf16
  - per query tile (512 wide): S^T chunk = K_chunk^T Q_tile -> PSUM,
    exp on ScalarE (scale folded in) -> E^T bf16, then accumulate
    O += V^T_chunk^T E^T and denom += ones^T E^T on TensorE.
    Normalize via reciprocal + ones-broadcast matmul, then Wo partial.
"""

import os
import numpy as np
import ml_dtypes

B, C, N, G = 2, 512, 4096, 4
GS = C // G          # 128 head dim
SCALE = GS ** -0.5
QT = 512             # query tile width
NQT = N // QT        # 8 query tiles
NKC = N // 128       # 32 key chunks
NCC = C // 128       # 4 contraction chunks for projections
NMC = C // 128       # 4 output-channel chunks

_compiled_nc = None
LAST_RESULT = None


def _build():
    from contextlib import ExitStack
    import concourse.mybir as mybir
    import concourse.tile as tile
    from concourse import bacc

    dt = mybir.dt
    f32 = dt.float32
    bf16 = dt.bfloat16
    Exp = mybir.ActivationFunctionType.Exp

    nc = bacc.Bacc("TRN2", target_bir_lowering=False, debug=False, num_devices=8)

    xb = nc.dram_tensor("xb", [C, N], bf16, kind="ExternalInput")
    wqT = nc.dram_tensor("wqT", [C, GS], bf16, kind="ExternalInput")
    wkT = nc.dram_tensor("wkT", [C, GS], bf16, kind="ExternalInput")
    wvT = nc.dram_tensor("wvT", [C, GS], bf16, kind="ExternalInput")
    woT = nc.dram_tensor("woT", [GS, C], bf16, kind="ExternalInput")
    bq = nc.dram_tensor("bq", [GS, 1], f32, kind="ExternalInput")
    bk = nc.dram_tensor("bk", [GS, 1], f32, kind="ExternalInput")
    bvb = nc.dram_tensor("bvb", [128, GS], f32, kind="ExternalInput")
    outp = nc.dram_tensor("outp", [C, N], f32, kind="ExternalOutput")

    KG = 2                    # key chunks per exp group
    GW = KG * 128             # exp group width in keys
    NGR = N // GW             # 16 groups per query tile

    with tile.TileContext(nc) as tc, ExitStack() as ctx:
        persist = ctx.enter_context(tc.tile_pool(name="persist", bufs=1))
        epool = ctx.enter_context(tc.tile_pool(name="epool", bufs=4))
        # bufs=4: tail chains of consecutive q-tiles otherwise couple
        # through slot reuse and cascade-delay each other
        spool = ctx.enter_context(tc.tile_pool(name="spool", bufs=4))
        # PSUM budget (8 banks): psS 2x[128,1024]=4, psO 2x[128,512]=2,
        # psD 1, psP 1.
        psS = ctx.enter_context(tc.tile_pool(name="psS", bufs=2, space="PSUM"))
        psO = ctx.enter_context(tc.tile_pool(name="psO", bufs=2, space="PSUM"))
        psD = ctx.enter_context(tc.tile_pool(name="psD", bufs=1, space="PSUM"))
        psP = ctx.enter_context(tc.tile_pool(name="psP", bufs=1, space="PSUM"))

        def load(shape, dtype, dram_ap, tag):
            t = persist.tile(shape, dtype, tag=tag)
            nc.sync.dma_start(t[:], dram_ap)
            return t

        # Weights first: they gate the first projection matmuls.
        wq = [load([128, GS], bf16, wqT[cc * 128:(cc + 1) * 128, :], f"wq{cc}")
              for cc in range(NCC)]
        wk = [load([128, GS], bf16, wkT[cc * 128:(cc + 1) * 128, :], f"wk{cc}")
              for cc in range(NCC)]
        wv = [load([128, GS], bf16, wvT[cc * 128:(cc + 1) * 128, :], f"wv{cc}")
              for cc in range(NCC)]
        wo_sb = load([GS, C], bf16, woT[:, :], "wo")
        bq_sb = load([GS, 1], f32, bq[:, :], "bq")
        bk_sb = load([GS, 1], f32, bk[:, :], "bk")
        bvb_sb = load([128, GS], f32, bvb[:, :], "bvb")

        xf = [[None] * NQT for _ in range(NCC)]

        def load_xf(nt):
            for cc in range(NCC):
                xf[cc][nt] = load(
                    [128, QT], bf16,
                    xb[cc * 128:(cc + 1) * 128, nt * QT:(nt + 1) * QT],
                    f"xf{cc}_{nt}")

        ones_k = persist.tile([128, 1], bf16, tag="ones_k")
        nc.vector.memset(ones_k[:], 1.0)
        ones_1 = persist.tile([1, 128], bf16, tag="ones_1")
        nc.vector.memset(ones_1[:], 1.0)

        q_sb = persist.tile([GS, N], bf16, tag="q_sb")
        k_sb = persist.tile([GS, N], bf16, tag="k_sb")
        vt_sb = persist.tile([128, N], bf16, tag="vt_sb")

        # Projections, emitted per column block (nt) with its xf loads
        # inline so compute starts after ~4 DMAs and overlaps the rest.
        # Q/K: [gs, N] = W_g @ xf (+ bias per partition);
        # V^T: [keys, gs] per 128-key chunk = xf_chunk^T @ Wv_g^T.
        load_xf(0)
        for nt in range(NQT):
            if nt + 1 < NQT:
                load_xf(nt + 1)
            nsl = slice(nt * QT, (nt + 1) * QT)
            for w_t, b_t, dst in ((wq, bq_sb, q_sb), (wk, bk_sb, k_sb)):
                ps = psO.tile([128, QT], f32, tag="po")
                for cc in range(NCC):
                    nc.tensor.matmul(ps[:], w_t[cc][:], xf[cc][nt][:],
                                     start=(cc == 0), stop=(cc == NCC - 1))
                nc.vector.tensor_scalar_add(dst[:, nsl], ps[:], b_t[:])
            for kc in range(nt * QT // 128, (nt + 1) * QT // 128):
                ksl = slice(kc * 128, (kc + 1) * 128)
                off = kc * 128 - nt * QT
                ps = psS.tile([128, GS], f32, tag="ps")
                for cc in range(NCC):
                    nc.tensor.matmul(ps[:], xf[cc][nt][:, off:off + 128],
                                     wv[cc][:],
                                     start=(cc == 0), stop=(cc == NCC - 1))
                nc.vector.tensor_add(vt_sb[:, ksl], ps[:], bvb_sb[:])

        # Attention, software-pipelined per query tile.
        # PE order per group g: S(g+1) matmuls, then O/D(g) — so PE stays
        # dense while ScalarE runs exp(g). Wide exp over KG key chunks.
        def emit_S(qt, g):
            qsl = slice(qt * QT, (qt + 1) * QT)
            ps = psS.tile([128, GW // 128 * QT], f32, tag="ps")
            for j in range(KG):
                kc = g * KG + j
                ksl = slice(kc * 128, (kc + 1) * 128)
                nc.tensor.matmul(ps[:, j * QT:(j + 1) * QT],
                                 k_sb[:, ksl], q_sb[:, qsl],
                                 start=True, stop=True)
            return ps

        tails = []

        def emit_tail(qt, po, pd):
            state = {}

            def tail_pre():
                # free the pd bank + start the reciprocal chain early
                den_sb = spool.tile([1, QT], bf16, tag="den")
                nc.vector.tensor_copy(den_sb[:], pd[:])
                pb = psP.tile([128, QT], f32, tag="pp")
                nc.tensor.matmul(pb[:], ones_1[:], den_sb[:],
                                 start=True, stop=True)
                binv = spool.tile([128, QT], f32, tag="binv")
                nc.vector.reciprocal(binv[:], pb[:])
                state["binv"] = binv

            def tail_main():
                qsl = slice(qt * QT, (qt + 1) * QT)
                o_sb = spool.tile([128, QT], bf16, tag="osb")
                nc.vector.tensor_mul(o_sb[:], po[:], state["binv"][:])
                for mc in range(NMC):
                    msl = slice(mc * 128, (mc + 1) * 128)
                    pp = psP.tile([128, QT], f32, tag="pp")
                    nc.tensor.matmul(pp[:], wo_sb[:, msl], o_sb[:],
                                     start=True, stop=True)
                    st = spool.tile([128, QT], f32, tag="st")
                    nc.vector.tensor_copy(st[:], pp[:])
                    nc.sync.dma_start(outp[msl, qsl], st[:])
            return tail_pre, tail_main

        for qt in range(NQT):
            po = psO.tile([128, QT], f32, tag="po")
            s_cur = emit_S(qt, 0)
            if tails:
                tails[-1][0]()         # prev epilogue: den copy + bcast + recip
            pd = psD.tile([1, QT], f32, tag="pd")
            for g in range(NGR):
                # rest of prev epilogue two groups in: by now its DVE
                # chain is done, so the Wo matmuls don't stall PE
                if g == 2 and tails:
                    tails.pop()[1]()
                s_next = emit_S(qt, g + 1) if g + 1 < NGR else None
                e = epool.tile([128, GW // 128 * QT], bf16, tag="e")
                nc.scalar.activation(e[:], s_cur[:], Exp, scale=SCALE)
                # same-psum-bank matmuls back-to-back: [O,O] then [D,D]
                # (interleaving accumulating matmuls across banks measured
                # ~1.5x slower per matmul)
                for j in range(KG):
                    kc = g * KG + j
                    ksl = slice(kc * 128, (kc + 1) * 128)
                    esl = slice(j * QT, (j + 1) * QT)
                    nc.tensor.matmul(po[:], vt_sb[:, ksl], e[:, esl],
                                     start=(kc == 0), stop=(kc == NKC - 1))
                for j in range(KG):
                    kc = g * KG + j
                    esl = slice(j * QT, (j + 1) * QT)
                    nc.tensor.matmul(pd[:], ones_k[:], e[:, esl],
                                     start=(kc == 0), stop=(kc == NKC - 1))
                s_cur = s_next
            tails.append(emit_tail(qt, po, pd))
        tp, tm = tails.pop()
        tp()
        tm()

    nc.compile()
    return nc


def _get_compiled():
    global _compiled_nc
    if _compiled_nc is None:
        _compiled_nc = _build()
    return _compiled_nc


def _ensure_ntff_hook():
    """Best-effort: register the axon NTFF profile hook so trace=True
    yields exec_time_ns. The image's antenv lacks axon_hooks; shim it."""
    import sys, types
    try:
        from antenv.axon_hooks import get_axon_ntff_profile_hook  # noqa: F401
        return
    except ImportError:
        pass
    try:
        mod = types.ModuleType("antenv.axon_hooks")
        _hook = [None]
        mod.set_axon_ntff_profile_hook = lambda h: _hook.__setitem__(0, h)
        mod.get_axon_ntff_profile_hook = lambda: _hook[0]
        sys.modules["antenv.axon_hooks"] = mod
        import antenv
        antenv.axon_hooks = mod
        from trn_agent_boot.trn_boot import _ntff_profile_via_ctypes
        mod.set_axon_ntff_profile_hook(
            _ntff_profile_via_ctypes("/opt/axon/libaxon_pjrt.so"))
    except Exception:
        pass


def kernel(x, Wq, bq, Wk, bk, Wv, bv, Wo, bo):
    global LAST_RESULT
    from concourse.bass_utils import run_bass_kernel_spmd

    nc = _get_compiled()
    bf = ml_dtypes.bfloat16
    x = np.asarray(x, dtype=np.float32)
    b, c, d, h, w = x.shape
    n = d * h * w
    xf = x.reshape(b, c, n)
    Wq = np.asarray(Wq, np.float32)
    Wk = np.asarray(Wk, np.float32)
    Wv = np.asarray(Wv, np.float32)
    Wo = np.asarray(Wo, np.float32)
    bq = np.asarray(bq, np.float32)
    bk = np.asarray(bk, np.float32)
    bv = np.asarray(bv, np.float32)
    bo = np.asarray(bo, np.float32)

    in_maps = []
    for core in range(8):
        bb, g = divmod(core, G)
        gsl = slice(g * GS, (g + 1) * GS)
        in_maps.append({
            "xb": np.ascontiguousarray(xf[bb]).astype(bf),
            "wqT": np.ascontiguousarray(Wq[gsl, :].T).astype(bf),
            "wkT": np.ascontiguousarray(Wk[gsl, :].T).astype(bf),
            "wvT": np.ascontiguousarray(Wv[gsl, :].T).astype(bf),
            "woT": np.ascontiguousarray(Wo[:, gsl].T).astype(bf),
            "bq": bq[gsl].reshape(GS, 1).copy(),
            "bk": bk[gsl].reshape(GS, 1).copy(),
            "bvb": np.ascontiguousarray(np.broadcast_to(bv[gsl], (128, GS))),
        })

    trace = bool(os.environ.get("BASS_TRACE"))
    if trace:
        _ensure_ntff_hook()
    LAST_RESULT = run_bass_kernel_spmd(
        nc, in_maps, core_ids=list(range(8)), trace=trace)
    outs = LAST_RESULT.results

    out = np.empty((b, c, n), np.float32)
    for bb in range(b):
        acc = xf[bb] + bo[:, None]
        for g in range(G):
            acc = acc + outs[bb * G + g]["outp"]
        out[bb] = acc
    return out.reshape(b, c, d, h, w)



# revision 9
# speedup vs baseline: 1.1993x; 1.1993x over previous
"""Grouped-query attention kernel for 8 Trainium2 NeuronCores.

Problem (hardcoded): x [2, 512, 16, 16, 16] f32, Wq/Wk/Wv/Wo [512, 512],
biases [512]. G=4 heads of dim 128, N=4096 tokens. out = x + Wo@attn.

Sharding: one (batch, group) pair per core -> 8 cores, no cross-core
communication. Each core computes its group's Q/K/V projections, the
full 4096x4096 attention for its (b, g), and a partial output
projection Wo[:, g_cols] @ O_g -> [512, 4096]. Host sums the 4 partials
per batch and adds the residual + bo.

Device-side layout (per core):
  - xf (x[b] as [512, 4096]) bf16, 4 chunks of [128, 4096] in SBUF
  - Q, K: [128(gs), 4096] bf16; V^T: [128(keys-chunk), 32*128] fp8e4
  - per query tile (512 wide): S^T chunk = K_chunk^T Q_tile -> PSUM,
    exp on ScalarE (scale + softmax-invariant -1.5 shift folded in)
    -> E^T fp8e4, then fp8 DoubleRow matmuls (contraction 256 = 2 key
    chunks per instruction) accumulate O += V^T E^T and
    denom += ones^T E^T on TensorE at 2x rate.
    Normalize via reciprocal + ones-broadcast matmul, then Wo partial.
"""

import os
import numpy as np
import ml_dtypes

B, C, N, G = 2, 512, 4096, 4
GS = C // G          # 128 head dim
SCALE = GS ** -0.5
QT = 512             # query tile width
NQT = N // QT        # 8 query tiles
NKC = N // 128       # 32 key chunks
NCC = C // 128       # 4 contraction chunks for projections
NMC = C // 128       # 4 output-channel chunks
ESHIFT = 1.5         # exp(s*scale - ESHIFT): softmax-invariant shift so
                     # max exp (~642) fits fp8e4's 448 ceiling

_compiled_nc = None
LAST_RESULT = None


def _build():
    from contextlib import ExitStack
    import concourse.mybir as mybir
    import concourse.tile as tile
    from concourse import bacc

    dt = mybir.dt
    f32 = dt.float32
    bf16 = dt.bfloat16
    f8 = dt.float8e4
    DR = mybir.MatmulPerfMode.DoubleRow
    Exp = mybir.ActivationFunctionType.Exp

    nc = bacc.Bacc("TRN2", target_bir_lowering=False, debug=False, num_devices=8)

    xb = nc.dram_tensor("xb", [C, N], bf16, kind="ExternalInput")
    wqT = nc.dram_tensor("wqT", [C, GS], bf16, kind="ExternalInput")
    wkT = nc.dram_tensor("wkT", [C, GS], bf16, kind="ExternalInput")
    wvT = nc.dram_tensor("wvT", [C, GS], bf16, kind="ExternalInput")
    woT = nc.dram_tensor("woT", [GS, C], bf16, kind="ExternalInput")
    bq = nc.dram_tensor("bq", [GS, 1], f32, kind="ExternalInput")
    bk = nc.dram_tensor("bk", [GS, 1], f32, kind="ExternalInput")
    bvb = nc.dram_tensor("bvb", [128, GS], f32, kind="ExternalInput")
    outp = nc.dram_tensor("outp", [C, N], f32, kind="ExternalOutput")

    KG = 2                    # key chunks per exp group
    GW = KG * 128             # exp group width in keys
    NGR = N // GW             # 16 groups per query tile

    with tile.TileContext(nc) as tc, ExitStack() as ctx:
        persist = ctx.enter_context(tc.tile_pool(name="persist", bufs=1))
        epool = ctx.enter_context(tc.tile_pool(name="epool", bufs=4))
        # bufs=4: tail chains of consecutive q-tiles otherwise couple
        # through slot reuse and cascade-delay each other
        spool = ctx.enter_context(tc.tile_pool(name="spool", bufs=4))
        # PSUM budget (8 banks): psS 2x[128,1024]=4, psO 2x[128,512]=2,
        # psD 1, psP 1.
        psS = ctx.enter_context(tc.tile_pool(name="psS", bufs=2, space="PSUM"))
        psO = ctx.enter_context(tc.tile_pool(name="psO", bufs=2, space="PSUM"))
        psD = ctx.enter_context(tc.tile_pool(name="psD", bufs=1, space="PSUM"))
        psP = ctx.enter_context(tc.tile_pool(name="psP", bufs=1, space="PSUM"))

        def load(shape, dtype, dram_ap, tag):
            t = persist.tile(shape, dtype, tag=tag)
            nc.sync.dma_start(t[:], dram_ap)
            return t

        # Weights first: they gate the first projection matmuls.
        wq = [load([128, GS], bf16, wqT[cc * 128:(cc + 1) * 128, :], f"wq{cc}")
              for cc in range(NCC)]
        wk = [load([128, GS], bf16, wkT[cc * 128:(cc + 1) * 128, :], f"wk{cc}")
              for cc in range(NCC)]
        wv = [load([128, GS], bf16, wvT[cc * 128:(cc + 1) * 128, :], f"wv{cc}")
              for cc in range(NCC)]
        wo_sb = load([GS, C], bf16, woT[:, :], "wo")
        bq_sb = load([GS, 1], f32, bq[:, :], "bq")
        bk_sb = load([GS, 1], f32, bk[:, :], "bk")
        bvb_sb = load([128, GS], f32, bvb[:, :], "bvb")

        xf = [[None] * NQT for _ in range(NCC)]

        def load_xf(nt):
            for cc in range(NCC):
                xf[cc][nt] = load(
                    [128, QT], bf16,
                    xb[cc * 128:(cc + 1) * 128, nt * QT:(nt + 1) * QT],
                    f"xf{cc}_{nt}")

        # fp8 DoubleRow D-matmul lhsT: [128, 2, 16] (k-tile step 16B for the
        # ISA perf-mode check) -> pd gets 16 identical denominator rows.
        ones_k = persist.tile([128, 32], f8, tag="ones_k")
        nc.vector.memset(ones_k[:], 1.0)
        ones_1 = persist.tile([1, 128], bf16, tag="ones_1")
        nc.vector.memset(ones_1[:], 1.0)
        eshift = persist.tile([128, 1], f32, tag="eshift")
        nc.vector.memset(eshift[:], -ESHIFT)

        q_sb = persist.tile([GS, N], bf16, tag="q_sb")
        k_sb = persist.tile([GS, N], bf16, tag="k_sb")
        vt_sb = persist.tile([128, N], f8, tag="vt_sb")

        # Projections, emitted per column block (nt) with its xf loads
        # inline so compute starts after ~4 DMAs and overlaps the rest.
        # Q/K: [gs, N] = W_g @ xf (+ bias per partition);
        # V^T: [keys, gs] per 128-key chunk = xf_chunk^T @ Wv_g^T.
        load_xf(0)
        for nt in range(NQT):
            if nt + 1 < NQT:
                load_xf(nt + 1)
            nsl = slice(nt * QT, (nt + 1) * QT)
            for w_t, b_t, dst in ((wq, bq_sb, q_sb), (wk, bk_sb, k_sb)):
                ps = psO.tile([128, QT], f32, tag="po")
                for cc in range(NCC):
                    nc.tensor.matmul(ps[:], w_t[cc][:], xf[cc][nt][:],
                                     start=(cc == 0), stop=(cc == NCC - 1))
                nc.vector.tensor_scalar_add(dst[:, nsl], ps[:], b_t[:])
            for kc in range(nt * QT // 128, (nt + 1) * QT // 128):
                ksl = slice(kc * 128, (kc + 1) * 128)
                off = kc * 128 - nt * QT
                ps = psS.tile([128, GS], f32, tag="ps")
                for cc in range(NCC):
                    nc.tensor.matmul(ps[:], xf[cc][nt][:, off:off + 128],
                                     wv[cc][:],
                                     start=(cc == 0), stop=(cc == NCC - 1))
                nc.vector.tensor_add(vt_sb[:, ksl], ps[:], bvb_sb[:])

        # Attention, software-pipelined per query tile.
        # PE order per group g: S(g+1) matmuls, then O/D(g) — so PE stays
        # dense while ScalarE runs exp(g). Wide exp over KG key chunks.
        def emit_S(qt, g):
            qsl = slice(qt * QT, (qt + 1) * QT)
            ps = psS.tile([128, GW // 128 * QT], f32, tag="ps")
            for j in range(KG):
                kc = g * KG + j
                ksl = slice(kc * 128, (kc + 1) * 128)
                nc.tensor.matmul(ps[:, j * QT:(j + 1) * QT],
                                 k_sb[:, ksl], q_sb[:, qsl],
                                 start=True, stop=True)
            return ps

        tails = []

        def emit_tail(qt, po, pd):
            state = {}

            def tail_pre():
                # free the pd bank + start the reciprocal chain early
                den_sb = spool.tile([1, QT], bf16, tag="den")
                nc.vector.tensor_copy(den_sb[:], pd[0:1, :])
                pb = psP.tile([128, QT], f32, tag="pp")
                nc.tensor.matmul(pb[:], ones_1[:], den_sb[:],
                                 start=True, stop=True)
                binv = spool.tile([128, QT], f32, tag="binv")
                nc.vector.reciprocal(binv[:], pb[:])
                state["binv"] = binv

            def tail_main():
                qsl = slice(qt * QT, (qt + 1) * QT)
                o_sb = spool.tile([128, QT], bf16, tag="osb")
                nc.vector.tensor_mul(o_sb[:], po[:], state["binv"][:])
                for mc in range(NMC):
                    msl = slice(mc * 128, (mc + 1) * 128)
                    pp = psP.tile([128, QT], f32, tag="pp")
                    nc.tensor.matmul(pp[:], wo_sb[:, msl], o_sb[:],
                                     start=True, stop=True)
                    st = spool.tile([128, QT], f32, tag="st")
                    nc.vector.tensor_copy(st[:], pp[:])
                    nc.sync.dma_start(outp[msl, qsl], st[:])
            return tail_pre, tail_main

        for qt in range(NQT):
            po = psO.tile([128, QT], f32, tag="po")
            s_cur = emit_S(qt, 0)
            if tails:
                tails[-1][0]()         # prev epilogue: den copy + bcast + recip
            pd = psD.tile([16, QT], f32, tag="pd")
            for g in range(NGR):
                # rest of prev epilogue two groups in: by now its DVE
                # chain is done, so the Wo matmuls don't stall PE
                if g == 2 and tails:
                    tails.pop()[1]()
                s_next = emit_S(qt, g + 1) if g + 1 < NGR else None
                e = epool.tile([128, GW // 128 * QT], f8, tag="e")
                nc.scalar.activation(e[:], s_cur[:], Exp,
                                     scale=SCALE, bias=eshift[:])
                # fp8 DoubleRow: both key chunks of the group in one
                # matmul ([128, 2, *] k-tiled APs, contraction 256)
                e3 = e[:].rearrange("p (t n) -> p t n", t=2)
                v3 = vt_sb[:, g * GW:(g + 1) * GW].rearrange(
                    "p (t m) -> p t m", t=2)
                o3 = ones_k[:].rearrange("p (t m) -> p t m", t=2)
                nc.tensor.matmul(po[:], v3, e3, perf_mode=DR,
                                 start=(g == 0), stop=(g == NGR - 1))
                nc.tensor.matmul(pd[:], o3, e3, perf_mode=DR,
                                 start=(g == 0), stop=(g == NGR - 1))
                s_cur = s_next
            tails.append(emit_tail(qt, po, pd))
        tp, tm = tails.pop()
        tp()
        tm()

    nc.compile()
    return nc


def _get_compiled():
    global _compiled_nc
    if _compiled_nc is None:
        _compiled_nc = _build()
    return _compiled_nc


def _ensure_ntff_hook():
    """Best-effort: register the axon NTFF profile hook so trace=True
    yields exec_time_ns. The image's antenv lacks axon_hooks; shim it."""
    import sys, types
    try:
        from antenv.axon_hooks import get_axon_ntff_profile_hook  # noqa: F401
        return
    except ImportError:
        pass
    try:
        mod = types.ModuleType("antenv.axon_hooks")
        _hook = [None]
        mod.set_axon_ntff_profile_hook = lambda h: _hook.__setitem__(0, h)
        mod.get_axon_ntff_profile_hook = lambda: _hook[0]
        sys.modules["antenv.axon_hooks"] = mod
        import antenv
        antenv.axon_hooks = mod
        from trn_agent_boot.trn_boot import _ntff_profile_via_ctypes
        mod.set_axon_ntff_profile_hook(
            _ntff_profile_via_ctypes("/opt/axon/libaxon_pjrt.so"))
    except Exception:
        pass


def kernel(x, Wq, bq, Wk, bk, Wv, bv, Wo, bo):
    global LAST_RESULT
    from concourse.bass_utils import run_bass_kernel_spmd

    nc = _get_compiled()
    bf = ml_dtypes.bfloat16
    x = np.asarray(x, dtype=np.float32)
    b, c, d, h, w = x.shape
    n = d * h * w
    xf = x.reshape(b, c, n)
    Wq = np.asarray(Wq, np.float32)
    Wk = np.asarray(Wk, np.float32)
    Wv = np.asarray(Wv, np.float32)
    Wo = np.asarray(Wo, np.float32)
    bq = np.asarray(bq, np.float32)
    bk = np.asarray(bk, np.float32)
    bv = np.asarray(bv, np.float32)
    bo = np.asarray(bo, np.float32)

    in_maps = []
    for core in range(8):
        bb, g = divmod(core, G)
        gsl = slice(g * GS, (g + 1) * GS)
        in_maps.append({
            "xb": np.ascontiguousarray(xf[bb]).astype(bf),
            "wqT": np.ascontiguousarray(Wq[gsl, :].T).astype(bf),
            "wkT": np.ascontiguousarray(Wk[gsl, :].T).astype(bf),
            "wvT": np.ascontiguousarray(Wv[gsl, :].T).astype(bf),
            "woT": np.ascontiguousarray(Wo[:, gsl].T).astype(bf),
            "bq": bq[gsl].reshape(GS, 1).copy(),
            "bk": bk[gsl].reshape(GS, 1).copy(),
            "bvb": np.ascontiguousarray(np.broadcast_to(bv[gsl], (128, GS))),
        })

    trace = bool(os.environ.get("BASS_TRACE"))
    if trace:
        _ensure_ntff_hook()
    LAST_RESULT = run_bass_kernel_spmd(
        nc, in_maps, core_ids=list(range(8)), trace=trace)
    outs = LAST_RESULT.results

    out = np.empty((b, c, n), np.float32)
    for bb in range(b):
        acc = xf[bb] + bo[:, None]
        for g in range(G):
            acc = acc + outs[bb * G + g]["outp"]
        out[bb] = acc
    return out.reshape(b, c, d, h, w)



# revision 10
# speedup vs baseline: 1.2477x; 1.0404x over previous
"""Grouped-query attention kernel for 8 Trainium2 NeuronCores.

Problem (hardcoded): x [2, 512, 16, 16, 16] f32, Wq/Wk/Wv/Wo [512, 512],
biases [512]. G=4 heads of dim 128, N=4096 tokens. out = x + Wo@attn.

Sharding: one (batch, group) pair per core -> 8 cores, no cross-core
communication. Each core computes its group's Q/K/V projections, the
full 4096x4096 attention for its (b, g), and a partial output
projection Wo[:, g_cols] @ O_g -> [512, 4096]. Host sums the 4 partials
per batch and adds the residual + bo.

Device-side layout (per core):
  - x chunk-pair tiles [128, 2048] fp8e4 per 512-token block (one
    batched DMA each, issued from the idle GpSimd queue)
  - Q, K: [128(gs), 4096] bf16 via fp8 DoubleRow projections
    (contraction 256 per instruction); V^T: [128(keys), 32*128] fp8e4
    via plain fp8 matmuls
  - attention: flat (qtile, group) pipeline with cross-qtile S
    lookahead so ScalarE (exp) never stalls at qtile boundaries.
    S^T chunk = K_chunk^T Q_tile (bf16) -> PSUM, exp on ScalarE
    (scale + softmax-invariant -1.5 shift folded in) -> E^T fp8e4,
    then fp8 DoubleRow matmuls (2 key chunks per instruction)
    accumulate O += V^T E^T and denom += ones^T E^T at 2x PE rate.
    Normalize via reciprocal + ones-broadcast matmul, then Wo partial.
"""

import os
import numpy as np
import ml_dtypes

B, C, N, G = 2, 512, 4096, 4
GS = C // G          # 128 head dim
SCALE = GS ** -0.5
QT = 512             # query tile width
NQT = N // QT        # 8 query tiles
NKC = N // 128       # 32 key chunks
NCC = C // 128       # 4 contraction chunks for projections
NMC = C // 128       # 4 output-channel chunks
ESHIFT = 1.5         # exp(s*scale - ESHIFT): softmax-invariant shift so
                     # max exp (~642) fits fp8e4's 448 ceiling
KG = 2               # key chunks per exp group (= DoubleRow pair)
GW = KG * 128        # group width in keys
NGR = N // GW        # 16 groups per query tile

_compiled_nc = None
LAST_RESULT = None


def _build():
    from contextlib import ExitStack
    import concourse.mybir as mybir
    import concourse.tile as tile
    from concourse import bacc

    dt = mybir.dt
    f32 = dt.float32
    bf16 = dt.bfloat16
    f8 = dt.float8e4
    DR = mybir.MatmulPerfMode.DoubleRow
    Exp = mybir.ActivationFunctionType.Exp

    nc = bacc.Bacc("TRN2", target_bir_lowering=False, debug=False, num_devices=8)

    xb8 = nc.dram_tensor("xb8", [C, N], f8, kind="ExternalInput")
    wq8d = nc.dram_tensor("wq8d", [C, GS], f8, kind="ExternalInput")
    wk8d = nc.dram_tensor("wk8d", [C, GS], f8, kind="ExternalInput")
    wv8d = nc.dram_tensor("wv8d", [C, GS], f8, kind="ExternalInput")
    woT = nc.dram_tensor("woT", [GS, C], bf16, kind="ExternalInput")
    bq = nc.dram_tensor("bq", [GS, 1], f32, kind="ExternalInput")
    bk = nc.dram_tensor("bk", [GS, 1], f32, kind="ExternalInput")
    bvb = nc.dram_tensor("bvb", [128, GS], f32, kind="ExternalInput")
    outp = nc.dram_tensor("outp", [C, N], f32, kind="ExternalOutput")

    with tile.TileContext(nc) as tc, ExitStack() as ctx:
        persist = ctx.enter_context(tc.tile_pool(name="persist", bufs=1))
        epool = ctx.enter_context(tc.tile_pool(name="epool", bufs=4))
        # bufs=4: tail chains of consecutive q-tiles otherwise couple
        # through slot reuse and cascade-delay each other
        spool = ctx.enter_context(tc.tile_pool(name="spool", bufs=4))
        # PSUM budget (8 banks): ps 2x[128,1024]=4, po 2x[128,512]=2,
        # pd 1, pp 1.
        psS = ctx.enter_context(tc.tile_pool(name="psS", bufs=2, space="PSUM"))
        psO = ctx.enter_context(tc.tile_pool(name="psO", bufs=2, space="PSUM"))
        psD = ctx.enter_context(tc.tile_pool(name="psD", bufs=1, space="PSUM"))
        psP = ctx.enter_context(tc.tile_pool(name="psP", bufs=1, space="PSUM"))

        # Weights first (they gate the first projections), then x blocks.
        # All input DMAs are batched (one per tensor / x block) and issued
        # from the GpSimd queue: the serial per-dma_start issue cost on the
        # sync sequencer was gating the whole projection phase.
        def wload(dram, tag):
            t = persist.tile([128, 4 * GS], f8, tag=tag)
            nc.gpsimd.dma_start(t[:].rearrange("p (c m) -> p c m", c=4),
                                dram[:, :].rearrange("(c p) m -> p c m", c=4))
            return t

        wq8 = wload(wq8d, "wq8")
        wk8 = wload(wk8d, "wk8")
        wv8 = wload(wv8d, "wv8")
        wo_sb = persist.tile([GS, C], bf16, tag="wo")
        nc.gpsimd.dma_start(wo_sb[:], woT[:, :])
        bq_sb = persist.tile([GS, 1], f32, tag="bq")
        nc.gpsimd.dma_start(bq_sb[:], bq[:, :])
        bk_sb = persist.tile([GS, 1], f32, tag="bk")
        nc.gpsimd.dma_start(bk_sb[:], bk[:, :])
        bvb_sb = persist.tile([128, GS], f32, tag="bvb")
        nc.gpsimd.dma_start(bvb_sb[:], bvb[:, :])

        xf8 = [None] * NQT
        for nt in range(NQT):
            t = persist.tile([128, 4 * QT], f8, tag=f"xf8_{nt}")
            nc.gpsimd.dma_start(
                t[:].rearrange("p (c n) -> p c n", c=4),
                xb8[:, nt * QT:(nt + 1) * QT].rearrange("(c p) n -> p c n", c=4))
            xf8[nt] = t

        # fp8 DoubleRow D-matmul lhsT: [128, 2, 16] (k-tile step 16B for
        # the ISA perf-mode check) -> pd gets 16 identical denom rows.
        ones_k = persist.tile([128, 32], f8, tag="ones_k")
        nc.vector.memset(ones_k[:], 1.0)
        ones_1 = persist.tile([1, 128], bf16, tag="ones_1")
        nc.vector.memset(ones_1[:], 1.0)
        eshift = persist.tile([128, 1], f32, tag="eshift")
        nc.vector.memset(eshift[:], -ESHIFT)

        q_sb = persist.tile([GS, N], bf16, tag="q_sb")
        k_sb = persist.tile([GS, N], bf16, tag="k_sb")
        vt_sb = persist.tile([128, N], f8, tag="vt_sb")

        # Projections. Q/K: fp8 DoubleRow, contraction 256 per matmul
        # (channel-chunk pairs); V^T: plain fp8 per 128-key chunk.
        for nt in range(NQT):
            nsl = slice(nt * QT, (nt + 1) * QT)
            xt = xf8[nt]
            for w8, b_t, dst in ((wq8, bq_sb, q_sb), (wk8, bk_sb, k_sb)):
                ps = psO.tile([128, QT], f32, tag="po")
                for j in range(2):
                    nc.tensor.matmul(
                        ps[:],
                        w8[:, j * 2 * GS:(j + 1) * 2 * GS].rearrange(
                            "p (t m) -> p t m", t=2),
                        xt[:, j * 2 * QT:(j + 1) * 2 * QT].rearrange(
                            "p (t n) -> p t n", t=2),
                        perf_mode=DR, start=(j == 0), stop=(j == 1))
                nc.vector.tensor_scalar_add(dst[:, nsl], ps[:], b_t[:])
            for idx in range(QT // 128):
                kc = nt * 4 + idx
                ksl = slice(kc * 128, (kc + 1) * 128)
                off = idx * 128
                ps = psS.tile([128, GS], f32, tag="ps")
                for cc in range(NCC):
                    j, t = divmod(cc, 2)
                    xsl = slice(j * 2 * QT + t * QT + off,
                                j * 2 * QT + t * QT + off + 128)
                    nc.tensor.matmul(ps[:], xt[:, xsl],
                                     wv8[:, cc * GS:(cc + 1) * GS],
                                     start=(cc == 0), stop=(cc == NCC - 1))
                nc.vector.tensor_add(vt_sb[:, ksl], ps[:], bvb_sb[:])

        # Attention: flat (qtile, group) stream with one-group lookahead
        # on S so exp(qt+1, 0) never waits at a qtile boundary.
        def emit_S(i):
            qt, g = divmod(i, NGR)
            qsl = slice(qt * QT, (qt + 1) * QT)
            ps = psS.tile([128, KG * QT], f32, tag="ps")
            for j in range(KG):
                kc = g * KG + j
                ksl = slice(kc * 128, (kc + 1) * 128)
                nc.tensor.matmul(ps[:, j * QT:(j + 1) * QT],
                                 k_sb[:, ksl], q_sb[:, qsl],
                                 start=True, stop=True)
            return ps

        tails = []

        def emit_tail(qt, po, pd, last=False):
            state = {}

            def tail_pre():
                # free the pd bank + start the reciprocal chain early
                den_sb = spool.tile([1, QT], bf16, tag="den")
                nc.vector.tensor_copy(den_sb[:], pd[0:1, :])
                pb = psP.tile([128, QT], f32, tag="pp")
                nc.tensor.matmul(pb[:], ones_1[:], den_sb[:],
                                 start=True, stop=True)
                binv = spool.tile([128, QT], f32, tag="binv")
                nc.vector.reciprocal(binv[:], pb[:])
                state["binv"] = binv

            def tail_main():
                qsl = slice(qt * QT, (qt + 1) * QT)
                o_sb = spool.tile([128, QT], bf16, tag="osb")
                nc.vector.tensor_mul(o_sb[:], po[:], state["binv"][:])
                for mc in range(NMC):
                    msl = slice(mc * 128, (mc + 1) * 128)
                    # last tail: alternate pp between the pp and (now
                    # free) pd banks to halve the serial epilogue
                    if last and mc % 2 == 1:
                        pp = psD.tile([128, QT], f32, tag="pd")
                    else:
                        pp = psP.tile([128, QT], f32, tag="pp")
                    nc.tensor.matmul(pp[:], wo_sb[:, msl], o_sb[:],
                                     start=True, stop=True)
                    st = spool.tile([128, QT], f32, tag="st")
                    nc.vector.tensor_copy(st[:], pp[:])
                    nc.gpsimd.dma_start(outp[msl, qsl], st[:])
            return tail_pre, tail_main

        NI = NQT * NGR
        po = pd = None
        s_cur = emit_S(0)
        for i in range(NI):
            qt, g = divmod(i, NGR)
            if g == 0:
                po = psO.tile([128, QT], f32, tag="po")
                pd = psD.tile([16, QT], f32, tag="pd")
            s_next = emit_S(i + 1) if i + 1 < NI else None
            if g == 0 and tails:
                tails[-1][0]()        # prev tail_pre: den copy + bcast + recip
            if g == 2 and tails:
                tails.pop()[1]()      # prev tail_main: normalize + Wo + store
            e = epool.tile([128, KG * QT], f8, tag="e")
            nc.scalar.activation(e[:], s_cur[:], Exp,
                                 scale=SCALE, bias=eshift[:])
            e3 = e[:].rearrange("p (t n) -> p t n", t=2)
            v3 = vt_sb[:, g * GW:(g + 1) * GW].rearrange("p (t m) -> p t m", t=2)
            o3 = ones_k[:].rearrange("p (t m) -> p t m", t=2)
            nc.tensor.matmul(po[:], v3, e3, perf_mode=DR,
                             start=(g == 0), stop=(g == NGR - 1))
            nc.tensor.matmul(pd[:], o3, e3, perf_mode=DR,
                             start=(g == 0), stop=(g == NGR - 1))
            if g == NGR - 1:
                tails.append(emit_tail(qt, po, pd, last=(i == NI - 1)))
            s_cur = s_next
        tp, tm = tails.pop()
        tp()
        tm()

    nc.compile()
    return nc


def _get_compiled():
    global _compiled_nc
    if _compiled_nc is None:
        _compiled_nc = _build()
    return _compiled_nc


def _ensure_ntff_hook():
    """Best-effort: register the axon NTFF profile hook so trace=True
    yields exec_time_ns. The image's antenv lacks axon_hooks; shim it."""
    import sys, types
    try:
        from antenv.axon_hooks import get_axon_ntff_profile_hook  # noqa: F401
        return
    except ImportError:
        pass
    try:
        mod = types.ModuleType("antenv.axon_hooks")
        _hook = [None]
        mod.set_axon_ntff_profile_hook = lambda h: _hook.__setitem__(0, h)
        mod.get_axon_ntff_profile_hook = lambda: _hook[0]
        sys.modules["antenv.axon_hooks"] = mod
        import antenv
        antenv.axon_hooks = mod
        from trn_agent_boot.trn_boot import _ntff_profile_via_ctypes
        mod.set_axon_ntff_profile_hook(
            _ntff_profile_via_ctypes("/opt/axon/libaxon_pjrt.so"))
    except Exception:
        pass


def kernel(x, Wq, bq, Wk, bk, Wv, bv, Wo, bo):
    global LAST_RESULT
    from concourse.bass_utils import run_bass_kernel_spmd

    nc = _get_compiled()
    bf = ml_dtypes.bfloat16
    f8 = ml_dtypes.float8_e4m3fn
    x = np.asarray(x, dtype=np.float32)
    b, c, d, h, w = x.shape
    n = d * h * w
    xf = x.reshape(b, c, n)
    Wq = np.asarray(Wq, np.float32)
    Wk = np.asarray(Wk, np.float32)
    Wv = np.asarray(Wv, np.float32)
    Wo = np.asarray(Wo, np.float32)
    bq = np.asarray(bq, np.float32)
    bk = np.asarray(bk, np.float32)
    bv = np.asarray(bv, np.float32)
    bo = np.asarray(bo, np.float32)

    in_maps = []
    for core in range(8):
        bb, g = divmod(core, G)
        gsl = slice(g * GS, (g + 1) * GS)
        in_maps.append({
            "xb8": np.ascontiguousarray(xf[bb]).astype(f8),
            "wq8d": np.ascontiguousarray(Wq[gsl, :].T).astype(f8),
            "wk8d": np.ascontiguousarray(Wk[gsl, :].T).astype(f8),
            "wv8d": np.ascontiguousarray(Wv[gsl, :].T).astype(f8),
            "woT": np.ascontiguousarray(Wo[:, gsl].T).astype(bf),
            "bq": bq[gsl].reshape(GS, 1).copy(),
            "bk": bk[gsl].reshape(GS, 1).copy(),
            "bvb": np.ascontiguousarray(np.broadcast_to(bv[gsl], (128, GS))),
        })

    trace = bool(os.environ.get("BASS_TRACE"))
    if trace:
        _ensure_ntff_hook()
    LAST_RESULT = run_bass_kernel_spmd(
        nc, in_maps, core_ids=list(range(8)), trace=trace)
    outs = LAST_RESULT.results

    out = np.empty((b, c, n), np.float32)
    for bb in range(b):
        acc = xf[bb] + bo[:, None]
        for g in range(G):
            acc = acc + outs[bb * G + g]["outp"]
        out[bb] = acc
    return out.reshape(b, c, d, h, w)
